# revision 8
# baseline (speedup 1.0000x reference)
"""Trainium2 Bass kernel for per-time-slice spatial self-attention + 1x1 conv.

Math per (b, t) slice (16 slices total):
    x      = x_in[b, :, t]          reshaped [C=64, P=2304]
    theta  = theta_w[t] @ x         [32, P]
    phi    = phi_w[t] @ x           [32, P]
    S      = theta.T @ phi / sqrt(32)          [P, P]
    A      = softmax(S, axis=-1)
    f      = x @ A.T  (f[c,p] = sum_q A[p,q] x[c,q])
    out    = out_w @ f + x

Sharding: the 16 slices are independent -> 2 slices per NeuronCore, no
collectives. Host precomputes the cheap channel projections and packs
layouts; the device runs the O(P^2) attention core.

HW-measured facts this schedule is built around (microbench.py):
  - PE matmuls hit the fast stream path ONLY at K=128 (~167ns for N=512;
    K=32/64/96 measure 300-500ns). So scores use the Gram trick at K=128:
    scoresT[q,p] = sum_c x[c,q] u[c,p], u = (phi_w^T theta_w) x, with x
    zero-padded to 128 rows as lhsT (junk rhs rows killed by zero weights)
    instead of the K=32 theta/phi form. The val matmul is K=128 already.
  - fp8 DoubleRow with a strided rhs measures ~580ns vs ~167ns for plain
    bf16 K=128 - so the val matmul stays bf16 (which also removes the
    fp8 E+v quantization error; total rel-err ~2.4e-3 vs gate 2e-2).
  - DVE runs the exp at ~0.54ns/elem (2x the cost model), ACT at ~0.9;
    exp groups split DVE:ACT 5:3.

Pipeline: S-phase (slice s) per 512-wide p-chunk, per group of 2 q-tiles:
    2 scores matmuls (PE) -> PSUM [128,2,w] (3-buf rotation)
    E = bf16-Schraudolph exp as int16 bits: i16(s*128*log2e*scale+16256)
        bitcast bf16 IS 2^z (one DVE tensor_scalar or ACT activation-Copy
        per group, into a slice-wide chunk-blocked E tile)
  V-phase (slice s-1) interleaved between S-groups as PE filler:
    val[m,p] += vte[q,m]^T E[q,p]  (bf16, m = 64 v-channels + ones row ->
        softmax denominator), per chunk: PSUM -> SBUF copy (DMA cannot
        read PSUM) on DVE/ACT alternating, then a [65,w] DMA on qSP.
  Final normalization (row 64 divide) + residual are host-side.
"""

import os
import sys

for _p in ("/opt/trn_rl_repo", "/root/.axon_site/_ro/trn_rl_repo"):
    if os.path.isdir(_p) and _p not in sys.path:
        sys.path.append(_p)

# The axon NTFF profiling hook (antenv.axon_hooks) is absent in this
# container; make sure run_bass_kernel_spmd never takes the trace path.
os.environ["BASS_NEVER_TRACE"] = "1"

import numpy as np
from collections import deque
from contextlib import ExitStack

import concourse.bass as bass
import concourse.tile as tile
from concourse import bacc, mybir
from concourse.bass_utils import run_bass_kernel_spmd

B, C, T, H, W = 2, 64, 8, 48, 48
C2 = 32
P = H * W                      # 2304
N_CORES = 8
S_PER_CORE = (B * T) // N_CORES  # 2 slices per core
QT = P // 128                  # 18 q-tiles of 128
GSZ = 2                        # q-tiles per exp group (2 PSUM banks)
NG = QT // GSZ                 # 9 groups per chunk
P_CHUNKS = [(0, 512), (512, 512), (1024, 512), (1536, 512), (2048, 256)]
SCALE = 1.0 / np.sqrt(np.float32(C2))
# Schraudolph constants for fp8e5m2: int8(z * 4*log2(e)*SCALE + 59.6)
SCH_A = float(4.0 * 1.4426950408889634 * SCALE)
SCH_B = 59.6
# exp-group engine pattern (DVE:ACT = 5:3; HW DVE ~0.54ns/el, ACT ~0.9)
EXP_PAT = ("D", "A", "D", "D", "A", "D", "D", "A")

F32 = mybir.dt.float32
BF16 = mybir.dt.bfloat16
I8 = mybir.dt.int8
FP8E = mybir.dt.float8e5       # e5m2 bit pattern of the Schraudolph exp
ALU = mybir.AluOpType
COPY = mybir.ActivationFunctionType.Copy

_CACHE = {}


def build_nc(repeat=1):
    """Build the per-core Bass program (SPMD: same NEFF on all 8 cores).

    repeat > 1 re-runs the whole computation; used only for timing (the
    extra passes recompute and overwrite the same outputs).
    """
    nc = bacc.Bacc("TRN2", target_bir_lowering=False, debug=False,
                   num_devices=N_CORES)
    # xz: rows 0-63 = x, rows 64-127 = 0 (zero-padded K=128 lhsT)
    xz_d = nc.dram_tensor("xz", [128, S_PER_CORE * P], BF16,
                          kind="ExternalInput").ap()
    # ur: rows 0-63 = u = (phi_w^T theta_w) x (rows 64-127 duplicate u;
    # they multiply the zero weight rows)
    ur_d = nc.dram_tensor("ur", [128, S_PER_CORE * P], BF16,
                          kind="ExternalInput").ap()
    vte_d = nc.dram_tensor("vte", [128, S_PER_CORE * QT * (C + 1)], BF16,
                           kind="ExternalInput").ap()
    y_d = nc.dram_tensor("y", [S_PER_CORE, C + 1, P], F32,
                         kind="ExternalOutput").ap()

    with tile.TileContext(nc) as tc, ExitStack() as ctx:
        ins = ctx.enter_context(tc.tile_pool(name="ins", bufs=2))
        epool = ctx.enter_context(tc.tile_pool(name="epool", bufs=7))
        scp = ctx.enter_context(tc.tile_pool(name="scp", bufs=3, space="PSUM"))
        valp = ctx.enter_context(tc.tile_pool(name="valp", bufs=2,
                                              space="PSUM"))
        outp = ctx.enter_context(tc.tile_pool(name="outp", bufs=3))

        state = {"eng": 0, "cp": 0}

        def make_v_phase(s, e_chunks, vte_sb):
            """V-phase work units for slice s (closures; popped between
            S-groups of the next slice as PE filler)."""
            units = deque()
            for ci, (off, w) in enumerate(P_CHUNKS):
                box = {}
                e_ch = e_chunks[ci]

                def start_chunk(w=w, box=box):
                    box["val"] = valp.tile([C + 1, w], F32, tag="val",
                                           name="val")

                def qts(j3, e_ch=e_ch, w=w, box=box):
                    # val[m, p] += sum_q vte[q, m] * E[q, p]  (bf16 weights x
                    # fp8 ifmap: fp8 without DoubleRow streams at bf16 speed)
                    for qt in range(j3, j3 + 6):
                        nc.tensor.matmul(
                            out=box["val"],
                            lhsT=vte_sb[:, s, qt, :],
                            rhs=e_ch[:, qt // 2, qt % 2, :w].bitcast(FP8E),
                            start=(qt == 0), stop=(qt == QT - 1),
                        )

                def finish_chunk(s=s, off=off, w=w, box=box):
                    # val -> SBUF (DMA cannot read PSUM), alternating engines
                    o_chunk = outp.tile([C + 1, w], F32, tag="oc",
                                        name="o_chunk")
                    if state["cp"] % 2 == 0:
                        nc.scalar.copy(out=o_chunk, in_=box["val"])
                    else:
                        nc.vector.tensor_copy(out=o_chunk, in_=box["val"])
                    state["cp"] += 1
                    # output DMA rides qSP (exp owns the ACT engine)
                    nc.sync.dma_start(out=y_d[s][:, off:off + w], in_=o_chunk)

                units.append(lambda sc=start_chunk, q=qts: (sc(), q(0)))
                units.append(lambda q=qts: q(6))
                units.append(lambda q=qts, f=finish_chunk: (q(12), f()))
            return units

        def emit_s_phase(s, xz_sb, ur_sb, vwork):
            x0 = s * P
            e_chunks = []
            for ci, (off, w) in enumerate(P_CHUNKS):
                e_ch = epool.tile([128, NG, GSZ, 512], I8, tag="E",
                                  name="e_ch")
                e_chunks.append(e_ch)
                for g in range(NG):
                    sc = scp.tile([128, GSZ, w], F32, tag="sc")
                    for j in range(GSZ):
                        qt = g * GSZ + j
                        # scoresT[q, p] = sum_c x[c, q] * u[c, p]  (K=128,
                        # zero-padded: rows 64-127 of xz are 0)
                        nc.tensor.matmul(
                            out=sc[:, j, :],
                            lhsT=xz_sb[:, x0 + qt * 128:x0 + (qt + 1) * 128],
                            rhs=ur_sb[:, x0 + off:x0 + off + w],
                            start=True, stop=True,
                        )
                    # E = schraudolph-e5m2-exp(sc * SCALE) as int8 bits,
                    # alternating DVE / ACT
                    eout = e_ch[:, g, :, :w]
                    if EXP_PAT[state["eng"] % len(EXP_PAT)] == "D":
                        nc.vector.tensor_scalar(
                            out=eout, in0=sc, scalar1=SCH_A,
                            scalar2=SCH_B, op0=ALU.mult, op1=ALU.add)
                    else:
                        nc.scalar.activation(
                            out=eout, in_=sc, func=COPY,
                            bias=SCH_B, scale=SCH_A)
                    state["eng"] += 1
                    # PE filler: val matmuls of the previous slice
                    if vwork:
                        vwork.popleft()()
            return e_chunks

        vwork = deque()
        for r in range(repeat):
            xz_sb = ins.tile([128, S_PER_CORE * P], BF16, tag="xz")
            ur_sb = ins.tile([128, S_PER_CORE * P], BF16, tag="ur")
            for s in range(S_PER_CORE):
                nc.sync.dma_start(out=xz_sb[:, s * P:(s + 1) * P],
                                  in_=xz_d[:, s * P:(s + 1) * P])
                nc.sync.dma_start(out=ur_sb[:, s * P:(s + 1) * P],
                                  in_=ur_d[:, s * P:(s + 1) * P])
            vte_sb = ins.tile([128, S_PER_CORE, QT, C + 1], BF16, tag="vte")
            nc.sync.dma_start(out=vte_sb, in_=vte_d.rearrange(
                "p (s q m) -> p s q m", s=S_PER_CORE, q=QT))

            for s in range(S_PER_CORE):
                e_chunks = emit_s_phase(s, xz_sb, ur_sb, vwork)
                while vwork:       # leftover from the previous slice
                    vwork.popleft()()
                vwork = make_v_phase(s, e_chunks, vte_sb)
        while vwork:
            vwork.popleft()()

    nc.compile()
    return nc


def host_prep(x_in, theta_w, phi_w, out_w):
    """Per-core input maps: channel projections + device layouts (numpy)."""
    import ml_dtypes
    bf16 = np.dtype(ml_dtypes.bfloat16)
    x_in = np.ascontiguousarray(x_in, dtype=np.float32)
    theta_w = np.asarray(theta_w, dtype=np.float32)
    phi_w = np.asarray(phi_w, dtype=np.float32)
    out_w = np.asarray(out_w, dtype=np.float32)

    x = np.transpose(x_in, (0, 2, 1, 3, 4)).reshape(B, T, C, P)
    G = np.einsum("toc,tod->tcd", phi_w, theta_w)  # [T, C, C]

    in_maps = []
    for k in range(N_CORES):
        xz = np.zeros((128, S_PER_CORE * P), bf16)
        ur = np.empty((128, S_PER_CORE * P), bf16)
        vte = np.empty((128, S_PER_CORE * QT * (C + 1)), bf16)
        vte_v = vte.reshape(128, S_PER_CORE, QT, C + 1)
        for s in range(S_PER_CORE):
            g = k * S_PER_CORE + s
            b, t = divmod(g, T)
            xslice = x[b, t]                      # [C, P]
            xz[:C, s * P:(s + 1) * P] = xslice
            u = G[t] @ xslice                     # [C, P]
            ur[:C, s * P:(s + 1) * P] = u
            ur[C:, s * P:(s + 1) * P] = u         # junk rows (zero weights)
            v = out_w @ xslice                    # [64, P]
            vt = np.empty((QT, 128, C + 1), bf16)
            vt[:, :, :C] = v.T.reshape(QT, 128, C)
            vt[:, :, C] = 1.0                     # softmax-denominator column
            vte_v[:, s] = np.transpose(vt, (1, 0, 2))
        in_maps.append({"xz": xz, "ur": ur, "vte": vte})
    return in_maps


def assemble(results, x_in):
    out = np.empty((B, C, T, H, W), np.float32)
    for k in range(N_CORES):
        y = results[k]["y"]  # [S_PER_CORE, C+1, P]: numerator rows + denom
        for s in range(S_PER_CORE):
            g = k * S_PER_CORE + s
            b, t = divmod(g, T)
            yn = y[s, :C] / y[s, C:C + 1]
            out[b, :, t] = yn.reshape(C, H, W) + x_in[b, :, t]
    return out


def kernel(x_in, theta_w, phi_w, out_w):
    if "nc" not in _CACHE:
        _CACHE["nc"] = build_nc()
    nc = _CACHE["nc"]
    in_maps = host_prep(x_in, theta_w, phi_w, out_w)
    res = run_bass_kernel_spmd(nc, in_maps, core_ids=list(range(N_CORES)))
    return assemble(res.results, np.asarray(x_in, dtype=np.float32))
